# revision 41
# baseline (speedup 1.0000x reference)
"""GCN layer (message passing) Trainium2 Bass kernel, 8-core SPMD.

Math: out = norm_dst * (segment_sum_dst(gather_src(norm_src * (h @ W)))) + b
Rewritten (matmul commutes with the linear aggregation):
    out = norm_dst * ((A @ (norm_src * h)) @ W) + b

Device strategy (per core c of 8; core owns dst rows [c*6250, (c+1)*6250)):
  - ALL inputs ship as ONE packed DRAM tensor `blob` [128, PITCH] u8 per
    core (per-execution runtime cost scales with the number of tensor
    bindings, ~36us each, dwarfing the compute).  Each partition row =
    [hg row bytes | gidx | mdst | norm_dst | W | bias | iota] with PITCH a
    multiple of 256B.
  - hg = (norm*h) cast bf16 in a grid layout: node n lives at blob row
    n%128, bytes [256*(n//128), +256).  A node's 256B gather cell index is
    CELLROW*(n%128) + n//128 where CELLROW=PITCH/256, kept < 32768 (int16)
    by a partition-half split (rows 0-63 / 64-127).
  - Host sorts edges by (dst window of 128 rows, src%128>=64); gpsimd
    dma_gather pulls hg[src] rows straight into bf16 SBUF tiles
    [128 edges x 128 feat] on 2 SWDGE queues - no on-device cast/scale.
  - DVE builds ALL one-hot S tiles of a superwindow in ONE is_equal op
    using stride-0 broadcast APs (iota columns vs per-tile dst-locals).
  - PE accumulates aggT[F, WIN] = sum_tiles msgs^T @ S per dst window;
    ACT copies aggT -> SBUF; PE projects out_w = aggT^T @ W (f32); DVE
    applies norm_dst and bias in one fused op; sync DMA stores per window.
"""
import os
import sys
import numpy as np

for _p in ("/opt/trn_rl_repo",):
    if _p not in sys.path and os.path.isdir(_p):
        sys.path.insert(0, _p)

import ml_dtypes

import concourse.bacc as bacc
import concourse.bass as bass
import concourse.mybir as mybir
from concourse.alu_op_type import AluOpType
from concourse.library_config import mlp as mlp_library

BF16 = ml_dtypes.bfloat16

# ---------------- problem constants (hardcoded per contract) ----------------
N, F, E, C = 50000, 128, 800000, 8
R = N // C                      # 6250 dst rows per core
WIN = 128                       # dst rows per window (PSUM partition dim)
NW = (R + WIN - 1) // WIN       # 49 windows per core
TILE = 128                      # edges per tile (PE contraction dim)
NCH = (N + 127) // 128          # 391 node chunks in the hg grid
NPAD = NCH * 128                # 50048
SW = 2                          # windows per superwindow
NSW = (NW + SW - 1) // SW       # 25 superwindows
GCHUNK = 8                      # tiles (x128 descs) per gather call; >1024 descs/call wedges
SCRATCH = 57344                 # SWDGE ring: 3584 descs so gen(k+1) overlaps transfer(k)


def _meta_layout(n_tiles):
    """Byte offsets of the metadata sections (after the hg region)."""
    off = {}
    pos = 0

    def sect(name, nbytes, align=4):
        nonlocal pos
        pos = (pos + align - 1) // align * align
        off[name] = pos
        pos += nbytes

    sect("gidx", n_tiles * 8 * 2)
    sect("mdst", n_tiles * 2)
    sect("nd", NW * 4)
    sect("W", F * 4)
    sect("bias", F * 4)
    sect("iota", WIN * 2)
    M = (pos + 255) // 256 * 256
    return off, M


# ---------------------------- host preprocessing ----------------------------

def host_prep(h, norm, W, b, src, dst):
    src = np.ascontiguousarray(np.asarray(src).astype(np.int64))
    dst = np.ascontiguousarray(np.asarray(dst).astype(np.int64))
    norm_f = np.asarray(norm, dtype=np.float32).reshape(-1)

    core = dst // R
    w_of = (dst % R) // WIN
    half = ((src % 128) >= 64).astype(np.int64)

    key = (core * NW + w_of) * 2 + half
    order = np.argsort(key, kind="stable")
    src_s, dst_s = src[order], dst[order]
    key_s = key[order]

    sizes = np.bincount(key_s, minlength=C * NW * 2).reshape(C, NW, 2)
    starts = np.zeros(C * NW * 2 + 1, dtype=np.int64)
    np.cumsum(sizes.reshape(-1), out=starts[1:])

    # tiles per (window, half): max over cores, both halves forced >= 1
    Tlo = np.maximum(1, -(-sizes[:, :, 0].max(axis=0) // TILE))
    Thi = np.maximum(1, -(-sizes[:, :, 1].max(axis=0) // TILE))

    sw_windows = [list(range(s * SW, min((s + 1) * SW, NW))) for s in range(NSW)]

    tile_window, tile_half = [], []
    call_list = []  # (sw, half, t0, n_tiles_in_call)
    sw_tile_start = {}
    for s in range(NSW):
        sw_tile_start[s] = len(tile_window)
        for hf in (0, 1):
            t0 = len(tile_window)
            for w in sw_windows[s]:
                T = int((Tlo if hf == 0 else Thi)[w])
                tile_window += [w] * T
                tile_half += [hf] * T
            call_list.append((s, hf, t0, len(tile_window) - t0))
    tile_window = np.asarray(tile_window)
    n_tiles = len(tile_window)
    sw_tile_start[NSW] = n_tiles

    first_tile, last_tile = {}, {}
    for t, w in enumerate(tile_window):
        w = int(w)
        first_tile.setdefault(w, t)
        last_tile[w] = t

    maxT_sw = max(sw_tile_start[s + 1] - sw_tile_start[s] for s in range(NSW))
    tile_off_in_sw = np.zeros(n_tiles, dtype=np.int64)
    for s in range(NSW):
        t0, t1 = sw_tile_start[s], sw_tile_start[s + 1]
        tile_off_in_sw[t0:t1] = np.arange(t1 - t0)

    moff, M = _meta_layout(n_tiles)
    PITCH = NPAD * 2 + M
    CELLROW = PITCH // 256

    pattern = dict(
        n_tiles=n_tiles, sw_windows=sw_windows, tile_window=tile_window,
        call_list=call_list, first_tile=first_tile, last_tile=last_tile,
        maxT_sw=maxT_sw, sw_tile_start=sw_tile_start,
        tile_off_in_sw=tile_off_in_sw, moff=moff, M=M, PITCH=PITCH,
        CELLROW=CELLROW,
    )

    # ---- shared pieces ----
    W_np = np.asarray(W, dtype=np.float32)
    b_np = np.asarray(b, dtype=np.float32).reshape(-1)
    bias_tile = np.tile(b_np[None, :], (128, 1)).astype(np.float32)
    iota_bf = np.tile(np.arange(WIN, dtype=np.float32)[None, :], (128, 1)).astype(BF16)
    h_np = np.asarray(h, dtype=np.float32)

    # hg grid: hg[p, c*128+f] = (norm*h)[128c+p, f], bf16
    h_ns = np.zeros((NPAD, 128), dtype=np.float32)
    h_ns[:N] = h_np * norm_f[:, None]
    hg = np.ascontiguousarray(
        h_ns.reshape(NCH, 128, 128).transpose(1, 0, 2).reshape(128, NPAD)
    ).astype(BF16)
    hg_bytes = hg.view(np.uint8)          # [128, NPAD*2]

    cores = []
    for c in range(C):
        gidx_flat = np.zeros(n_tiles * TILE, dtype=np.int16)
        mdst_flat = np.full(n_tiles * TILE, -1.0, dtype=np.float32)
        for s, hf, t0, ntc in call_list:
            pos = t0 * TILE
            for w in sw_windows[s]:
                g = (c * NW + w) * 2 + hf
                st, en = int(starts[g]), int(starts[g + 1])
                n = en - st
                T = int((Tlo if hf == 0 else Thi)[w])
                sl = slice(pos, pos + n)
                gsrc = src_s[st:en]
                gidx_flat[sl] = (
                    CELLROW * (gsrc % 128 - 64 * hf) + gsrc // 128
                ).astype(np.int16)
                mdst_flat[sl] = (dst_s[st:en] - c * R - w * WIN).astype(np.float32)
                pos += T * TILE

        # gather idx wrapped layout per call: [16, n/16] blocks, tiled x8
        blocks = []
        for s, hf, t0, ntc in call_list:
            fl = gidx_flat[t0 * TILE:(t0 + ntc) * TILE]
            blocks.append(fl.reshape(-1, 16).T)            # [16, ntc*8]
        gidx_wrapped = np.ascontiguousarray(
            np.tile(np.concatenate(blocks, axis=1), (8, 1)))

        mdst_t = np.ascontiguousarray(
            mdst_flat.reshape(n_tiles, TILE).T.astype(BF16))  # [128, n_tiles]

        nd = np.zeros((WIN, NW), dtype=np.float32)
        for w in range(NW):
            lo = c * R + w * WIN
            hi = min(lo + WIN, (c + 1) * R)
            nd[: hi - lo, w] = norm_f[lo:hi]

        blob = np.zeros((128, PITCH), dtype=np.uint8)
        blob[:, :NPAD * 2] = hg_bytes
        meta = blob[:, NPAD * 2:]

        def put(name, arr):
            bts = np.ascontiguousarray(arr).view(np.uint8)
            meta[:, moff[name]: moff[name] + bts.shape[1]] = bts

        put("gidx", gidx_wrapped)
        put("mdst", mdst_t)
        put("nd", nd)
        put("W", W_np)
        put("bias", bias_tile)
        put("iota", iota_bf)

        cores.append({"blob": blob})
    return cores, pattern


# ----------------------------- device program -------------------------------

def build_program(pat, gdepth=None, nqueues=None):
    GD = int(os.environ.get("GDEPTH", "4")) if gdepth is None else gdepth
    NQ = int(os.environ.get("NQUEUES", "2")) if nqueues is None else nqueues
    n_tiles = pat["n_tiles"]
    maxT = pat["maxT_sw"]
    tile_window = pat["tile_window"]
    first_tile, last_tile = pat["first_tile"], pat["last_tile"]
    call_list = pat["call_list"]
    sw_windows = pat["sw_windows"]
    sw_tile_start = pat["sw_tile_start"]
    tile_off = pat["tile_off_in_sw"]
    moff, M, PITCH = pat["moff"], pat["M"], pat["PITCH"]

    def sw_tiles(s):
        return list(range(sw_tile_start[s], sw_tile_start[s + 1]))

    # ---- PE op order & counters: per sw: tiles, then W-matmuls of prev sw ----
    pe_count_after = {}
    cnt = 0
    for s in range(NSW + 1):
        if s < NSW:
            for t in sw_tiles(s):
                cnt += 1
                pe_count_after[("tile", t)] = cnt
        if s >= 1:
            for w in sw_windows[s - 1]:
                cnt += 1
                pe_count_after[("wmm", w)] = cnt
    tiles_through_sw = {s: pe_count_after[("tile", sw_tiles(s)[-1])]
                        for s in range(NSW)}

    # ---- gather calls chunked by GCHUNK; per-sw-slot cumulative gsem ----
    chunks = []
    for s, hf, t0, ntc in call_list:
        for c0 in range(0, ntc, GCHUNK):
            chunks.append((s, hf, t0 + c0, min(GCHUNK, ntc - c0)))
    sw_gsem_target = {}
    _cum = {i: 0 for i in range(GD)}
    for s, hf, tstart, nt in chunks:
        _cum[s % GD] += 16
        sw_gsem_target[s] = _cum[s % GD]

    dt = mybir.dt
    nc = bacc.Bacc("TRN2", debug=False, dynamic_dma_scratch_size=SCRATCH,
                   num_swdge_queues=NQ)

    blob_d = nc.dram_tensor("blob", [128, PITCH], dt.uint8, kind="ExternalInput")
    out_d = nc.dram_tensor("out", [NW * WIN, F], dt.float32, kind="ExternalOutput")

    sb_meta = nc.alloc_sbuf_tensor("sb_meta", [128, M], dt.uint8)
    gbuf = nc.alloc_sbuf_tensor("gbuf", [128, GD, maxT, TILE], dt.bfloat16)
    sbuf_S = nc.alloc_sbuf_tensor("sbuf_S", [128, GD, maxT, WIN], dt.bfloat16)
    aggTs = nc.alloc_sbuf_tensor("aggTs", [F, 2, WIN], dt.float32)
    outsb = nc.alloc_sbuf_tensor("outsb", [WIN, 2, F], dt.float32)

    ps_agg = nc.alloc_psum_tensor("ps_agg", [128, 4, 512], dt.float32)
    ps_out = nc.alloc_psum_tensor("ps_out", [128, 2, 512], dt.float32)

    ld = nc.alloc_semaphore("ld")
    gsem = [nc.alloc_semaphore(f"gsem{i}") for i in range(GD)]
    sS = nc.alloc_semaphore("sS")
    pe_c = nc.alloc_semaphore("pe_c")
    aggc = nc.alloc_semaphore("aggc")
    dvsc = nc.alloc_semaphore("dvsc")
    osem = [nc.alloc_semaphore("osem0"), nc.alloc_semaphore("osem1")]

    # ---- typed views into the metadata blob ----
    def meta_view(name, nbytes, dtype):
        return sb_meta[:, moff[name]: moff[name] + nbytes].bitcast(dtype)

    def v_gidx(tstart, nt):
        a = moff["gidx"] + tstart * 8 * 2
        return sb_meta[:, a: a + nt * 8 * 2].bitcast(dt.int16)

    def v_nd(w):
        a = moff["nd"] + 4 * w
        return sb_meta[:, a: a + 4].bitcast(dt.float32)

    v_W = meta_view("W", F * 4, dt.float32)          # [128, F]
    v_bias = meta_view("bias", F * 4, dt.float32)    # [128, F]

    def v_iota_bcast(T):
        v = sb_meta[:, moff["iota"]: moff["iota"] + WIN * 2].bitcast(dt.bfloat16)
        return AP(v.tensor, v.offset, [[M // 2, 128], [0, T], [1, WIN]])

    def v_mdst_bcast(t0, T):
        a = moff["mdst"] + 2 * t0
        v = sb_meta[:, a: a + 2].bitcast(dt.bfloat16)
        return AP(v.tensor, v.offset, [[M // 2, 128], [1, T], [0, WIN]])

    N_LOADS = 1
    AP = bass.AP

    with nc.Block() as block:

        @block.sync
        def _(sync: bass.BassEngine):
            sync.dma_start(sb_meta[:, :],
                           blob_d[:, NPAD * 2: NPAD * 2 + M]).then_inc(ld, 16)
            for w in range(NW):
                sync.wait_ge(dvsc, w + 1)
                if w >= 2:
                    sync.wait_ge(osem[w % 2], 16 * (w // 2))
                sync.dma_start(
                    out_d[w * WIN:(w + 1) * WIN, :], outsb[:, w % 2, :]
                ).then_inc(osem[w % 2], 16)

        @block.gpsimd
        def _(gp: bass.BassGpSimd):
            gp.load_library(mlp_library)
            gp.wait_ge(ld, 16 * N_LOADS)
            blob_full = blob_d[:, :]
            _ablate = bool(os.environ.get("ABLATE_GATHER"))
            seen_sw = set()
            for s, hf, tstart, nt in chunks:
                if s not in seen_sw:
                    seen_sw.add(s)
                    if s >= GD:
                        gp.wait_ge(pe_c, tiles_through_sw[s - GD])
                if _ablate:
                    gp.sem_inc(gsem[s % GD], 16)
                    continue
                n_idx = nt * TILE
                off = int(tile_off[tstart])
                in_u8 = AP(blob_full.tensor, hf * 64 * PITCH,
                           [[256, 64 * (PITCH // 256)], [1, 256]])
                gp.dma_gather(
                    gbuf[:, s % GD, off:off + nt, :],
                    in_u8.bitcast(dt.bfloat16),
                    v_gidx(tstart, nt),
                    n_idx,
                    n_idx,
                    F,
                    queue_num=((s % GD) % NQ),
                ).then_inc(gsem[s % GD], 16)

        @block.tensor
        def _(pe):
            pe.wait_ge(ld, 16 * N_LOADS)
            for s in range(NSW + 1):
                if s < NSW:
                    for t in sw_tiles(s):
                        w = int(tile_window[t])
                        if first_tile[w] == t and w >= 4:
                            pe.wait_ge(aggc, w - 3)
                        if t == sw_tile_start[s]:
                            pe.wait_ge(sS, s + 1)
                            pe.wait_ge(gsem[s % GD], sw_gsem_target[s])
                        j = int(tile_off[t])
                        pe.matmul(
                            ps_agg[:, w % 4, 0:WIN],
                            gbuf[:, s % GD, j, :],
                            sbuf_S[:, s % GD, j, :],
                            start=(first_tile[w] == t),
                            stop=(last_tile[w] == t),
                        ).then_inc(pe_c)
                if s >= 1:
                    for w in sw_windows[s - 1]:
                        pe.wait_ge(aggc, w + 1)
                        if w >= 2:
                            pe.wait_ge(dvsc, w - 1)
                        pe.matmul(
                            ps_out[:, w % 2, 0:F],
                            aggTs[:, w % 2, :],
                            v_W,
                            start=True,
                            stop=True,
                        ).then_inc(pe_c)

        @block.scalar
        def _(act):
            act.wait_ge(ld, 16 * N_LOADS)
            for w in range(NW):
                tgt = pe_count_after[("tile", last_tile[w])]
                if w >= 2:
                    tgt = max(tgt, pe_count_after[("wmm", w - 2)])
                act.wait_ge(pe_c, tgt)
                act.activation(
                    aggTs[:, w % 2, :],
                    ps_agg[:, w % 4, 0:WIN],
                    mybir.ActivationFunctionType.Copy,
                ).then_inc(aggc)

        @block.vector
        def _(dve):
            dve.wait_ge(ld, 16 * N_LOADS)

            def build_S(s):
                t0, t1 = sw_tile_start[s], sw_tile_start[s + 1]
                T = t1 - t0
                if s >= GD:
                    dve.wait_ge(pe_c, tiles_through_sw[s - GD])
                dve.scalar_tensor_tensor(
                    sbuf_S[:, s % GD, 0:T, :],
                    v_iota_bcast(T),
                    0.0,
                    v_mdst_bcast(t0, T),
                    AluOpType.add,
                    AluOpType.is_equal,
                ).then_inc(sS)

            for _s0 in range(min(GD, NSW)):
                build_S(_s0)
            for s in range(NSW):
                for w in sw_windows[s]:
                    dve.wait_ge(pe_c, pe_count_after[("wmm", w)])
                    if w >= 2:
                        dve.wait_ge(osem[w % 2], 16 * (w // 2))
                    dve.scalar_tensor_tensor(
                        outsb[:, w % 2, :],
                        ps_out[:, w % 2, 0:F],
                        v_nd(w),
                        v_bias,
                        AluOpType.mult,
                        AluOpType.add,
                    ).then_inc(dvsc)
                if s + GD < NSW:
                    build_S(s + GD)

    nc.compile()
    return nc


# ------------------------------- entry point --------------------------------

def kernel(h, norm, W, b, src, dst):
    cores, pat = host_prep(h, norm, W, b, src, dst)
    nc = build_program(pat)

    from concourse.bass_utils import run_bass_kernel_spmd
    res = run_bass_kernel_spmd(nc, cores, core_ids=list(range(C)))
    outs = [res.results[c]["out"][:R] for c in range(C)]
    return np.ascontiguousarray(np.concatenate(outs, axis=0).astype(np.float32))


# revision 42
# speedup vs baseline: 1.0453x; 1.0453x over previous
"""GCN layer (message passing) Trainium2 Bass kernel, 8-core SPMD.

Math: out = norm_dst * (segment_sum_dst(gather_src(norm_src * (h @ W)))) + b
Rewritten (matmul commutes with the linear aggregation):
    out = norm_dst * ((A @ (norm_src * h)) @ W) + b

Device strategy (per core c of 8; core owns dst rows [c*6250, (c+1)*6250)):
  - ALL inputs ship as ONE packed DRAM tensor `blob` [128, PITCH] u8 per
    core (per-execution runtime cost scales with the number of tensor
    bindings, ~36us each, dwarfing the compute).  Each partition row =
    [hg row bytes | gidx | mdst | norm_dst | W | bias | iota] with PITCH a
    multiple of 256B.
  - hg = (norm*h) cast bf16 in a grid layout: node n lives at blob row
    n%128, bytes [256*(n//128), +256).  A node's 256B gather cell index is
    CELLROW*(n%128) + n//128 where CELLROW=PITCH/256, kept < 32768 (int16)
    by a partition-half split (rows 0-63 / 64-127).
  - Host sorts edges by (dst window of 128 rows, src%128>=64); gpsimd
    dma_gather pulls hg[src] rows straight into bf16 SBUF tiles
    [128 edges x 128 feat] on 2 SWDGE queues - no on-device cast/scale.
  - DVE builds ALL one-hot S tiles of a superwindow in ONE is_equal op
    using stride-0 broadcast APs (iota columns vs per-tile dst-locals).
  - PE accumulates aggT[F, WIN] = sum_tiles msgs^T @ S per dst window;
    ACT copies aggT -> SBUF; PE projects out_w = aggT^T @ W (f32); DVE
    applies norm_dst and bias in one fused op; sync DMA stores per window.
"""
import os
import sys
import numpy as np

for _p in ("/opt/trn_rl_repo",):
    if _p not in sys.path and os.path.isdir(_p):
        sys.path.insert(0, _p)

import ml_dtypes

import concourse.bacc as bacc
import concourse.bass as bass
import concourse.mybir as mybir
from concourse.alu_op_type import AluOpType
from concourse.library_config import mlp as mlp_library

BF16 = ml_dtypes.bfloat16

# ---------------- problem constants (hardcoded per contract) ----------------
N, F, E, C = 50000, 128, 800000, 8
R = N // C                      # 6250 dst rows per core
WIN = 128                       # dst rows per window (PSUM partition dim)
NW = (R + WIN - 1) // WIN       # 49 windows per core
TILE = 128                      # edges per tile (PE contraction dim)
NCH = (N + 127) // 128          # 391 node chunks in the hg grid
NPAD = NCH * 128                # 50048
SW = 2                          # windows per superwindow
NSW = (NW + SW - 1) // SW       # 25 superwindows
GCHUNK = 8                      # tiles (x128 descs) per gather call; >1024 descs/call wedges
SCRATCH = 57344                 # SWDGE ring: 3584 descs so gen(k+1) overlaps transfer(k)


def _meta_layout(n_tiles):
    """Byte offsets of the metadata sections (after the hg region)."""
    off = {}
    pos = 0

    def sect(name, nbytes, align=4):
        nonlocal pos
        pos = (pos + align - 1) // align * align
        off[name] = pos
        pos += nbytes

    sect("gidx", n_tiles * 8 * 2)
    sect("mdst", n_tiles * 2)
    sect("nd", NW * 4)
    sect("W", F * 4)
    sect("bias", F * 4)
    sect("iota", WIN * 2)
    M = (pos + 255) // 256 * 256
    return off, M


# ---------------------------- host preprocessing ----------------------------

def host_prep(h, norm, W, b, src, dst):
    src = np.ascontiguousarray(np.asarray(src).astype(np.int64))
    dst = np.ascontiguousarray(np.asarray(dst).astype(np.int64))
    norm_f = np.asarray(norm, dtype=np.float32).reshape(-1)

    core = dst // R
    w_of = (dst % R) // WIN
    half = ((src % 128) >= 64).astype(np.int64)

    key = (core * NW + w_of) * 2 + half
    order = np.argsort(key, kind="stable")
    src_s, dst_s = src[order], dst[order]
    key_s = key[order]

    sizes = np.bincount(key_s, minlength=C * NW * 2).reshape(C, NW, 2)
    starts = np.zeros(C * NW * 2 + 1, dtype=np.int64)
    np.cumsum(sizes.reshape(-1), out=starts[1:])

    # tiles per (window, half): max over cores, both halves forced >= 1
    Tlo = np.maximum(1, -(-sizes[:, :, 0].max(axis=0) // TILE))
    Thi = np.maximum(1, -(-sizes[:, :, 1].max(axis=0) // TILE))

    sw_windows = [list(range(s * SW, min((s + 1) * SW, NW))) for s in range(NSW)]

    tile_window, tile_half = [], []
    call_list = []  # (sw, half, t0, n_tiles_in_call)
    sw_tile_start = {}
    for s in range(NSW):
        sw_tile_start[s] = len(tile_window)
        for hf in (0, 1):
            t0 = len(tile_window)
            for w in sw_windows[s]:
                T = int((Tlo if hf == 0 else Thi)[w])
                tile_window += [w] * T
                tile_half += [hf] * T
            call_list.append((s, hf, t0, len(tile_window) - t0))
    tile_window = np.asarray(tile_window)
    n_tiles = len(tile_window)
    sw_tile_start[NSW] = n_tiles

    first_tile, last_tile = {}, {}
    for t, w in enumerate(tile_window):
        w = int(w)
        first_tile.setdefault(w, t)
        last_tile[w] = t

    maxT_sw = max(sw_tile_start[s + 1] - sw_tile_start[s] for s in range(NSW))
    tile_off_in_sw = np.zeros(n_tiles, dtype=np.int64)
    for s in range(NSW):
        t0, t1 = sw_tile_start[s], sw_tile_start[s + 1]
        tile_off_in_sw[t0:t1] = np.arange(t1 - t0)

    moff, M = _meta_layout(n_tiles)
    PITCH = NPAD * 2 + M
    CELLROW = PITCH // 256

    pattern = dict(
        n_tiles=n_tiles, sw_windows=sw_windows, tile_window=tile_window,
        call_list=call_list, first_tile=first_tile, last_tile=last_tile,
        maxT_sw=maxT_sw, sw_tile_start=sw_tile_start,
        tile_off_in_sw=tile_off_in_sw, moff=moff, M=M, PITCH=PITCH,
        CELLROW=CELLROW,
    )

    # ---- shared pieces ----
    W_np = np.asarray(W, dtype=np.float32)
    b_np = np.asarray(b, dtype=np.float32).reshape(-1)
    bias_tile = np.tile(b_np[None, :], (128, 1)).astype(np.float32)
    iota_bf = np.tile(np.arange(WIN, dtype=np.float32)[None, :], (128, 1)).astype(BF16)
    h_np = np.asarray(h, dtype=np.float32)

    # hg grid: hg[p, c*128+f] = (norm*h)[128c+p, f], bf16
    h_ns = np.zeros((NPAD, 128), dtype=np.float32)
    h_ns[:N] = h_np * norm_f[:, None]
    hg = np.ascontiguousarray(
        h_ns.reshape(NCH, 128, 128).transpose(1, 0, 2).reshape(128, NPAD)
    ).astype(BF16)
    hg_bytes = hg.view(np.uint8)          # [128, NPAD*2]

    cores = []
    for c in range(C):
        gidx_flat = np.zeros(n_tiles * TILE, dtype=np.int16)
        mdst_flat = np.full(n_tiles * TILE, -1.0, dtype=np.float32)
        for s, hf, t0, ntc in call_list:
            pos = t0 * TILE
            for w in sw_windows[s]:
                g = (c * NW + w) * 2 + hf
                st, en = int(starts[g]), int(starts[g + 1])
                n = en - st
                T = int((Tlo if hf == 0 else Thi)[w])
                sl = slice(pos, pos + n)
                gsrc = src_s[st:en]
                gidx_flat[sl] = (
                    CELLROW * (gsrc % 128 - 64 * hf) + gsrc // 128
                ).astype(np.int16)
                mdst_flat[sl] = (dst_s[st:en] - c * R - w * WIN).astype(np.float32)
                pos += T * TILE

        # gather idx wrapped layout per call: [16, n/16] blocks, tiled x8
        blocks = []
        for s, hf, t0, ntc in call_list:
            fl = gidx_flat[t0 * TILE:(t0 + ntc) * TILE]
            blocks.append(fl.reshape(-1, 16).T)            # [16, ntc*8]
        gidx_wrapped = np.ascontiguousarray(
            np.tile(np.concatenate(blocks, axis=1), (8, 1)))

        mdst_t = np.ascontiguousarray(
            mdst_flat.reshape(n_tiles, TILE).T.astype(BF16))  # [128, n_tiles]

        nd = np.zeros((WIN, NW), dtype=np.float32)
        for w in range(NW):
            lo = c * R + w * WIN
            hi = min(lo + WIN, (c + 1) * R)
            nd[: hi - lo, w] = norm_f[lo:hi]

        blob = np.zeros((128, PITCH), dtype=np.uint8)
        blob[:, :NPAD * 2] = hg_bytes
        meta = blob[:, NPAD * 2:]

        def put(name, arr):
            bts = np.ascontiguousarray(arr).view(np.uint8)
            meta[:, moff[name]: moff[name] + bts.shape[1]] = bts

        put("gidx", gidx_wrapped)
        put("mdst", mdst_t)
        put("nd", nd)
        put("W", W_np)
        put("bias", bias_tile)
        put("iota", iota_bf)

        cores.append({"blob": blob})
    return cores, pattern


# ----------------------------- device program -------------------------------

def build_program(pat, gdepth=4, nqueues=2):
    GD = gdepth
    NQ = nqueues
    n_tiles = pat["n_tiles"]
    maxT = pat["maxT_sw"]
    tile_window = pat["tile_window"]
    first_tile, last_tile = pat["first_tile"], pat["last_tile"]
    call_list = pat["call_list"]
    sw_windows = pat["sw_windows"]
    sw_tile_start = pat["sw_tile_start"]
    tile_off = pat["tile_off_in_sw"]
    moff, M, PITCH = pat["moff"], pat["M"], pat["PITCH"]

    def sw_tiles(s):
        return list(range(sw_tile_start[s], sw_tile_start[s + 1]))

    # ---- PE op order & counters: per sw: tiles, then W-matmuls of prev sw ----
    pe_count_after = {}
    cnt = 0
    for s in range(NSW + 1):
        if s < NSW:
            for t in sw_tiles(s):
                cnt += 1
                pe_count_after[("tile", t)] = cnt
        if s >= 1:
            for w in sw_windows[s - 1]:
                cnt += 1
                pe_count_after[("wmm", w)] = cnt
    tiles_through_sw = {s: pe_count_after[("tile", sw_tiles(s)[-1])]
                        for s in range(NSW)}

    # ---- gather calls chunked by GCHUNK; per-sw-slot cumulative gsem ----
    chunks = []
    for s, hf, t0, ntc in call_list:
        for c0 in range(0, ntc, GCHUNK):
            chunks.append((s, hf, t0 + c0, min(GCHUNK, ntc - c0)))
    sw_gsem_target = {}
    _cum = {i: 0 for i in range(GD)}
    for s, hf, tstart, nt in chunks:
        _cum[s % GD] += 16
        sw_gsem_target[s] = _cum[s % GD]

    dt = mybir.dt
    nc = bacc.Bacc("TRN2", debug=False, dynamic_dma_scratch_size=SCRATCH,
                   num_swdge_queues=NQ)

    blob_d = nc.dram_tensor("blob", [128, PITCH], dt.uint8, kind="ExternalInput")
    out_d = nc.dram_tensor("out", [NW * WIN, F], dt.float32, kind="ExternalOutput")

    sb_meta = nc.alloc_sbuf_tensor("sb_meta", [128, M], dt.uint8)
    gbuf = nc.alloc_sbuf_tensor("gbuf", [128, GD, maxT, TILE], dt.bfloat16)
    sbuf_S = nc.alloc_sbuf_tensor("sbuf_S", [128, GD, maxT, WIN], dt.bfloat16)
    aggTs = nc.alloc_sbuf_tensor("aggTs", [F, 2, WIN], dt.float32)
    outsb = nc.alloc_sbuf_tensor("outsb", [WIN, 2, F], dt.float32)

    ps_agg = nc.alloc_psum_tensor("ps_agg", [128, 4, 512], dt.float32)
    ps_out = nc.alloc_psum_tensor("ps_out", [128, 2, 512], dt.float32)

    ld = nc.alloc_semaphore("ld")
    gsem = [nc.alloc_semaphore(f"gsem{i}") for i in range(GD)]
    sS = nc.alloc_semaphore("sS")
    pe_c = nc.alloc_semaphore("pe_c")
    aggc = nc.alloc_semaphore("aggc")
    dvsc = nc.alloc_semaphore("dvsc")
    osem = [nc.alloc_semaphore("osem0"), nc.alloc_semaphore("osem1")]

    # ---- typed views into the metadata blob ----
    def meta_view(name, nbytes, dtype):
        return sb_meta[:, moff[name]: moff[name] + nbytes].bitcast(dtype)

    def v_gidx(tstart, nt):
        a = moff["gidx"] + tstart * 8 * 2
        return sb_meta[:, a: a + nt * 8 * 2].bitcast(dt.int16)

    def v_nd(w):
        a = moff["nd"] + 4 * w
        return sb_meta[:, a: a + 4].bitcast(dt.float32)

    v_W = meta_view("W", F * 4, dt.float32)          # [128, F]
    v_bias = meta_view("bias", F * 4, dt.float32)    # [128, F]

    def v_iota_bcast(T):
        v = sb_meta[:, moff["iota"]: moff["iota"] + WIN * 2].bitcast(dt.bfloat16)
        return AP(v.tensor, v.offset, [[M // 2, 128], [0, T], [1, WIN]])

    def v_mdst_bcast(t0, T):
        a = moff["mdst"] + 2 * t0
        v = sb_meta[:, a: a + 2].bitcast(dt.bfloat16)
        return AP(v.tensor, v.offset, [[M // 2, 128], [1, T], [0, WIN]])

    N_LOADS = 1
    AP = bass.AP

    with nc.Block() as block:

        @block.sync
        def _(sync: bass.BassEngine):
            sync.dma_start(sb_meta[:, :],
                           blob_d[:, NPAD * 2: NPAD * 2 + M]).then_inc(ld, 16)
            for w in range(NW):
                sync.wait_ge(dvsc, w + 1)
                if w >= 2:
                    sync.wait_ge(osem[w % 2], 16 * (w // 2))
                sync.dma_start(
                    out_d[w * WIN:(w + 1) * WIN, :], outsb[:, w % 2, :]
                ).then_inc(osem[w % 2], 16)

        @block.gpsimd
        def _(gp: bass.BassGpSimd):
            gp.load_library(mlp_library)
            gp.wait_ge(ld, 16 * N_LOADS)
            blob_full = blob_d[:, :]
            seen_sw = set()
            for s, hf, tstart, nt in chunks:
                if s not in seen_sw:
                    seen_sw.add(s)
                    if s >= GD:
                        gp.wait_ge(pe_c, tiles_through_sw[s - GD])
                n_idx = nt * TILE
                off = int(tile_off[tstart])
                in_u8 = AP(blob_full.tensor, hf * 64 * PITCH,
                           [[256, 64 * (PITCH // 256)], [1, 256]])
                gp.dma_gather(
                    gbuf[:, s % GD, off:off + nt, :],
                    in_u8.bitcast(dt.bfloat16),
                    v_gidx(tstart, nt),
                    n_idx,
                    n_idx,
                    F,
                    queue_num=((s % GD) % NQ),
                ).then_inc(gsem[s % GD], 16)

        @block.tensor
        def _(pe):
            pe.wait_ge(ld, 16 * N_LOADS)
            for s in range(NSW + 1):
                if s < NSW:
                    for t in sw_tiles(s):
                        w = int(tile_window[t])
                        if first_tile[w] == t and w >= 4:
                            pe.wait_ge(aggc, w - 3)
                        if t == sw_tile_start[s]:
                            pe.wait_ge(sS, s + 1)
                            pe.wait_ge(gsem[s % GD], sw_gsem_target[s])
                        j = int(tile_off[t])
                        pe.matmul(
                            ps_agg[:, w % 4, 0:WIN],
                            gbuf[:, s % GD, j, :],
                            sbuf_S[:, s % GD, j, :],
                            start=(first_tile[w] == t),
                            stop=(last_tile[w] == t),
                        ).then_inc(pe_c)
                if s >= 1:
                    for w in sw_windows[s - 1]:
                        pe.wait_ge(aggc, w + 1)
                        if w >= 2:
                            pe.wait_ge(dvsc, w - 1)
                        pe.matmul(
                            ps_out[:, w % 2, 0:F],
                            aggTs[:, w % 2, :],
                            v_W,
                            start=True,
                            stop=True,
                        ).then_inc(pe_c)

        @block.scalar
        def _(act):
            act.wait_ge(ld, 16 * N_LOADS)
            for w in range(NW):
                tgt = pe_count_after[("tile", last_tile[w])]
                if w >= 2:
                    tgt = max(tgt, pe_count_after[("wmm", w - 2)])
                act.wait_ge(pe_c, tgt)
                act.activation(
                    aggTs[:, w % 2, :],
                    ps_agg[:, w % 4, 0:WIN],
                    mybir.ActivationFunctionType.Copy,
                ).then_inc(aggc)

        @block.vector
        def _(dve):
            dve.wait_ge(ld, 16 * N_LOADS)

            def build_S(s):
                t0, t1 = sw_tile_start[s], sw_tile_start[s + 1]
                T = t1 - t0
                if s >= GD:
                    dve.wait_ge(pe_c, tiles_through_sw[s - GD])
                dve.scalar_tensor_tensor(
                    sbuf_S[:, s % GD, 0:T, :],
                    v_iota_bcast(T),
                    0.0,
                    v_mdst_bcast(t0, T),
                    AluOpType.add,
                    AluOpType.is_equal,
                ).then_inc(sS)

            for _s0 in range(min(GD, NSW)):
                build_S(_s0)
            for s in range(NSW):
                for w in sw_windows[s]:
                    dve.wait_ge(pe_c, pe_count_after[("wmm", w)])
                    if w >= 2:
                        dve.wait_ge(osem[w % 2], 16 * (w // 2))
                    dve.scalar_tensor_tensor(
                        outsb[:, w % 2, :],
                        ps_out[:, w % 2, 0:F],
                        v_nd(w),
                        v_bias,
                        AluOpType.mult,
                        AluOpType.add,
                    ).then_inc(dvsc)
                if s + GD < NSW:
                    build_S(s + GD)

    nc.compile()
    return nc


# ------------------------------- entry point --------------------------------

def kernel(h, norm, W, b, src, dst):
    cores, pat = host_prep(h, norm, W, b, src, dst)
    nc = build_program(pat)

    from concourse.bass_utils import run_bass_kernel_spmd
    res = run_bass_kernel_spmd(nc, cores, core_ids=list(range(C)))
    outs = [res.results[c]["out"][:R] for c in range(C)]
    return np.ascontiguousarray(np.concatenate(outs, axis=0).astype(np.float32))
